# revision 27
# baseline (speedup 1.0000x reference)
"""Trainium2 Bass kernel for nn_L2PppMaskAttn (topk_masking).

Math reformulation of the reference:
  - top-5 ranking over prompts is invariant to q normalization, so scores
    u[b,p] = <x[b,l], K_hat[l,p]> suffice; mask = (u >= 5th_max(u)).
  - a_k depends only on (layer, prompt): s[l,p] = <K_hat[l,p], A_hat[l,p]>.
  - out[l,b] = (mask_row .* s) @ P_flat[l]: a [128,100] @ [100,6144] matmul.

K_hat and s are computed at pack time in f64 and rounded to f32 once.  The
top-5 selection then reduces to 6 f32 PE matmuls per 128-row group against
the prepacked K_hat^T plus a DVE max8 threshold (comparison-only, exact).
The f64-true top-5 agrees with the reference's choice on every row of this
input set (min 5th/6th gap 9.5e-7 vs device f32 score noise ~2e-7), so the
selection reproduces the reference's mask.  The output-scale path (s, P
matmul, store) runs in bf16: ~5e-3 worst case vs the 2e-2 gate.

Sharding: work = 12 layers x 1024 rows, cut into 96 (layer, 128-row)
groups; each core takes 12 groups spanning just TWO layers (one full layer
= 8 groups + one half layer = 4 groups), so each layer's K_hat/P pool is
read by at most two cores instead of eight.  Per-core HBM traffic:
  kt    [128dd, (2 lay, j, p)]        f32   0.6 MB
  x     [128dd, (12 grp, j, b)]       f32   4.7 MB
  P     [2][100p, 6144]               bf16  2.5 MB
  out   [12 grp][128b, 6144]          bf16 18.9 MB
~26.7 MB/core vs ~42 MB for batch-only sharding (P/kt no longer 8x
replicated); the host reassembles groups into the [L, B, Lp, E] output.

Schedule: all loads ride one HWDGE ring (sync) in FIFO order (kt, then x
groups with the two P tiles slotted in), so group g's selection inputs
land well before its out matmuls need P.  The PE queue is software-
pipelined: selection for group g+1 is emitted before the 12 out matmuls of
group g, hiding selection latency inside the out stream.  Out matmuls
write 6 single-bank PSUM tiles; each 512-col tile is cast PSUM->SBUF bf16
on scalar or vector, and each group stores as two 0.79 MB halves on the
scalar HWDGE ring.
"""

import sys

sys.path.insert(0, "/opt/trn_rl_repo")

import numpy as np

B, L, P_N, LP, D = 1024, 12, 100, 8, 768
N_CORES = 8
BS = 128  # rows per group
NG = 12  # groups per core
NFULL = 8  # groups 0..7 -> layer A (full batch); 8..11 -> layer B (half)
NF = LP * D  # 6144 flattened output features per layer
NCH = D // 128  # 6 contraction chunks
C = NCH * P_N  # 600 K_hat^T columns per layer
TOP_K = 5

_CACHE = {}


def _layer_of(g):
    return 0 if g < NFULL else 1


def _core_layers(c):
    # core c: full layer c, half (c % 2) of layer 8 + c // 2
    return c, 8 + c // 2, c % 2


def _build_nc():
    if "nc" in _CACHE:
        return _CACHE["nc"]

    from contextlib import ExitStack

    import concourse.bass as bass
    import concourse.bacc as bacc
    import concourse.mybir as mybir
    from concourse import masks
    from concourse.tile import TileContext

    f32 = mybir.dt.float32
    bf16 = mybir.dt.bfloat16
    OP = mybir.AluOpType

    nc = bacc.Bacc(
        "TRN2",
        target_bir_lowering=False,
        debug=False,
        num_devices=N_CORES,
    )

    # one packed stream: [kt_A | x_0 | kt_B | x_1 | x_2 .. x_11]
    XW = 2 * C + NG * D
    x_d = nc.declare_dram_parameter("x", [128, XW], f32, isOutput=False)
    s_d = nc.declare_dram_parameter("s", [P_N, 2], f32, isOutput=False)
    p_d = nc.declare_dram_parameter("p", [2, P_N, NF], bf16, isOutput=False)
    o_d = nc.declare_dram_parameter("o", [NG, BS, NF], bf16, isOutput=True)

    with TileContext(nc) as tc, ExitStack() as ctx:
        pool = lambda name, bufs, **kw: ctx.enter_context(
            tc.tile_pool(name=name, bufs=bufs, **kw)
        )
        const = pool("const", 1)
        rowp = pool("rowp", 2)
        small = pool("small", 2)
        wtp = pool("wtp", 3)
        obuf = pool("ob", 4)
        ps_pc = pool("ps_pc", 1, space="PSUM")
        ps_mt = pool("ps_mt", 1, space="PSUM")
        ps_o = pool("ps_o", 3, space="PSUM")

        ident = const.tile([128, 128], bf16, tag="ident")
        masks.make_identity(nc, ident[:])

        s_all = const.tile([P_N, 2], f32, tag="s")
        nc.scalar.dma_start(s_all[:], s_d[:])

        x_all = const.tile([128, XW], f32, tag="x")
        p_A = const.tile([P_N, NF], bf16, tag="pA")
        p_B = const.tile([P_N, NF], bf16, tag="pB")

        # packed-stream offsets: [kt_A | x_0 | kt_B | x_1 | x_2 .. x_11]
        ktoff = lambda lay: 0 if lay == 0 else C + D
        xoff = lambda g: C if g == 0 else 2 * C + g * D

        # ---- all loads on one HWDGE ring, in need-order ----
        def _load(c0, c1):
            nc.sync.dma_start(x_all[:, c0:c1], x_d[:, c0:c1])

        _load(0, C + D)  # kt_A + x_0
        h2 = NF // 2  # P_A in halves so group 0's first matmuls start early
        nc.sync.dma_start(p_A[:, :h2], p_d[0, :, :h2])
        nc.sync.dma_start(p_A[:, h2:], p_d[0, :, h2:])
        _load(C + D, 2 * C + 2 * D)  # kt_B + x_1
        nc.sync.dma_start(p_B[:], p_d[1])
        for k in range(5):  # x_2..x_11 in 2-group chunks
            _load(2 * C + (2 + 2 * k) * D, 2 * C + (4 + 2 * k) * D)

        # ---- selection for one group: scores -> mask -> W^T ----
        wt_tiles = {}

        def _sel(g):
            lay = _layer_of(g)
            ktc = ktoff(lay)
            xc = xoff(g)
            pc = ps_pc.tile([BS, P_N], f32, tag="pc")
            for j in range(NCH):
                nc.tensor.matmul(
                    pc[:],
                    x_all[:, xc + j * 128 : xc + (j + 1) * 128],
                    x_all[:, ktc + j * P_N : ktc + (j + 1) * P_N],
                    start=(j == 0),
                    stop=(j == NCH - 1),
                )
            mm8 = small.tile([BS, 8], f32, tag="mm8")
            nc.vector.max(mm8[:], pc[:])
            mask = rowp.tile([BS, P_N], bf16, tag="mask")
            nc.vector.tensor_scalar(
                mask[:], pc[:], mm8[:, TOP_K - 1 : TOP_K], None, OP.is_ge
            )
            mt = ps_mt.tile([P_N, BS], bf16, tag="mt")
            nc.tensor.transpose(mt[:], mask[:], ident[:])
            wt = wtp.tile([P_N, BS], bf16, tag="wt")
            nc.vector.tensor_scalar_mul(wt[:], mt[:], s_all[:, lay : lay + 1])
            wt_tiles[g] = wt

        # ---- group loop: sel runs one group ahead of the out stream ----
        _sel(0)
        for g in range(NG):
            if g + 1 < NG:
                _sel(g + 1)
            wt = wt_tiles.pop(g)
            p_sb = p_A if _layer_of(g) == 0 else p_B
            ob = obuf.tile([BS, NF], bf16, tag="ob")
            for n in range(6):
                po = ps_o.tile([BS, 1024], f32, tag="po")
                for h in range(2):
                    nc.tensor.matmul(
                        po[:, h * 512 : (h + 1) * 512],
                        wt[:],
                        p_sb[:, (2 * n + h) * 512 : (2 * n + h + 1) * 512],
                        start=True,
                        stop=True,
                    )
                if n % 3 == 2:
                    nc.vector.tensor_copy(ob[:, n * 1024 : (n + 1) * 1024], po[:])
                else:
                    nc.scalar.copy(ob[:, n * 1024 : (n + 1) * 1024], po[:])
                if n % 2 == 1:
                    t = NF // 3
                    k = n // 2
                    nc.scalar.dma_start(
                        o_d[g, :, k * t : (k + 1) * t], ob[:, k * t : (k + 1) * t]
                    )

    nc.compile()
    _CACHE["nc"] = nc
    return nc


def _pack_inputs(x_query, K_all, A_all, P_all):
    import ml_dtypes

    bf = ml_dtypes.bfloat16
    x = np.asarray(x_query, dtype=np.float32)
    K64 = np.asarray(K_all, dtype=np.float64)
    A64 = np.asarray(A_all, dtype=np.float64)
    P = np.asarray(P_all, dtype=np.float32)

    Kh64 = K64 / np.linalg.norm(K64, axis=-1, keepdims=True)
    Ah64 = A64 / np.linalg.norm(A64, axis=-1, keepdims=True)
    Kh = Kh64.astype(np.float32)
    s = np.sum(Kh64 * Ah64, axis=-1).astype(np.float32)  # [L, P]
    pp = np.ascontiguousarray(P.reshape(L, P_N, NF)).astype(bf)

    # K_hat^T per layer: [128dd, (j, p)]
    kt6 = np.ascontiguousarray(
        Kh.reshape(L, P_N, NCH, 128).transpose(0, 3, 2, 1)
    )  # [L, 128, 6, 100]

    XW = 2 * C + NG * D
    xs, ss, ps = [], [], []
    for c in range(N_CORES):
        la, lb, hb = _core_layers(c)
        ss.append(np.stack([s[la], s[lb]], axis=1))  # [P, 2]
        ps.append(np.stack([pp[la], pp[lb]], axis=0))  # [2, P, NF]
        xg = np.empty((128, NG, NCH, 128), dtype=np.float32)
        for g in range(NG):
            if g < NFULL:
                lay, r0 = la, g * BS
            else:
                lay, r0 = lb, hb * 512 + (g - NFULL) * BS
            # x rows [r0:r0+128] of layer lay -> [128dd, (j, b)]
            xg[:, g] = x[r0 : r0 + BS, lay, :].reshape(BS, NCH, 128).transpose(2, 1, 0)
        xg = xg.reshape(128, NG, D)
        # packed stream: [kt_A | x_0 | kt_B | x_1 | x_2 .. x_11]
        xp = np.empty((128, XW), dtype=np.float32)
        xp[:, :C] = kt6[la].reshape(128, C)
        xp[:, C : C + D] = xg[:, 0]
        xp[:, C + D : 2 * C + D] = kt6[lb].reshape(128, C)
        xp[:, 2 * C + D :] = xg[:, 1:].reshape(128, (NG - 1) * D)
        xs.append(xp)
    return xs, ss, ps


def _run(x_query, K_all, A_all, P_all, trace=False, tmpdir=None):
    from concourse.bass_utils import run_bass_kernel_spmd

    xs, ss, ps = _pack_inputs(x_query, K_all, A_all, P_all)
    nc = _build_nc()
    in_maps = [{"x": xs[c], "s": ss[c], "p": ps[c]} for c in range(N_CORES)]
    br = run_bass_kernel_spmd(
        nc, in_maps, list(range(N_CORES)), trace=trace, tmpdir=tmpdir
    )
    out = np.empty((L, B, NF), dtype=np.float32)
    for c in range(N_CORES):
        o = np.asarray(br.results[c]["o"]).astype(np.float32)  # [NG, BS, NF]
        la, lb, hb = _core_layers(c)
        for g in range(NG):
            if g < NFULL:
                lay, r0 = la, g * BS
            else:
                lay, r0 = lb, hb * 512 + (g - NFULL) * BS
            out[lay, r0 : r0 + BS] = o[g]
    return out.reshape(L, B, LP, D), br


def kernel(x_query, K_all, A_all, P_all):
    out, _ = _run(x_query, K_all, A_all, P_all)
    return out


# revision 28
# speedup vs baseline: 1.1630x; 1.1630x over previous
"""Trainium2 Bass kernel for nn_L2PppMaskAttn (topk_masking).

Math reformulation of the reference:
  - top-5 ranking over prompts is invariant to q normalization, so scores
    u[b,p] = <x[b,l], K_hat[l,p]> suffice; mask = (u >= 5th_max(u)).
  - a_k depends only on (layer, prompt): s[l,p] = <K_hat[l,p], A_hat[l,p]>.
  - out[l,b] = (mask_row .* s) @ P_flat[l]: a [128,100] @ [100,6144] matmul.

K_hat and s are computed at pack time in f64 and rounded to f32 once.  The
top-5 selection then reduces to 6 f32 PE matmuls per 128-row group against
the prepacked K_hat^T plus a DVE max8 threshold (comparison-only, exact).
The f64-true top-5 agrees with the reference's choice on every row of this
input set (min 5th/6th gap 9.5e-7 vs device f32 score noise ~2e-7), so the
selection reproduces the reference's mask.  The output-scale path (s, P
matmul, store) runs in bf16: ~5e-3 worst case vs the 2e-2 gate.

Sharding: work = 12 layers x 1024 rows, cut into 96 (layer, 128-row)
groups; each core takes 12 groups spanning just TWO layers (one full layer
= 8 groups + one half layer = 4 groups), so each layer's K_hat/P pool is
read by at most two cores instead of eight.  Per-core HBM traffic:
  kt    [128dd, (2 lay, j, p)]        f32   0.6 MB
  x     [128dd, (12 grp, j, b)]       f32   4.7 MB
  P     [2][100p, 6144]               bf16  2.5 MB
  out   [12 grp][128b, 6144]          bf16 18.9 MB
~26.7 MB/core vs ~42 MB for batch-only sharding (P/kt no longer 8x
replicated); the host reassembles groups into the [L, B, Lp, E] output.

Schedule: all loads ride one HWDGE ring (sync) in FIFO order (kt, then x
groups with the two P tiles slotted in), so group g's selection inputs
land well before its out matmuls need P.  The PE queue is software-
pipelined: selection for group g+1 is emitted before the 12 out matmuls of
group g, hiding selection latency inside the out stream.  Out matmuls
write 6 single-bank PSUM tiles; each 512-col tile is cast PSUM->SBUF bf16
on scalar or vector, and each group stores as two 0.79 MB halves on the
scalar HWDGE ring.
"""

import sys

sys.path.insert(0, "/opt/trn_rl_repo")

import numpy as np

B, L, P_N, LP, D = 1024, 12, 100, 8, 768
N_CORES = 8
BS = 128  # rows per group
NG = 12  # groups per core
NFULL = 8  # groups 0..7 -> layer A (full batch); 8..11 -> layer B (half)
NF = LP * D  # 6144 flattened output features per layer
NCH = D // 128  # 6 contraction chunks
C = NCH * P_N  # 600 K_hat^T columns per layer
TOP_K = 5

_CACHE = {}


def _layer_of(g):
    return 0 if g < NFULL else 1


def _core_layers(c):
    # core c: full layer c, half (c % 2) of layer 8 + c // 2
    return c, 8 + c // 2, c % 2


def _build_nc():
    if "nc" in _CACHE:
        return _CACHE["nc"]

    from contextlib import ExitStack

    import concourse.bass as bass
    import concourse.bacc as bacc
    import concourse.mybir as mybir
    from concourse import masks
    from concourse.tile import TileContext

    f32 = mybir.dt.float32
    bf16 = mybir.dt.bfloat16
    OP = mybir.AluOpType

    nc = bacc.Bacc(
        "TRN2",
        target_bir_lowering=False,
        debug=False,
        num_devices=N_CORES,
    )

    # one packed stream: [kt_A | x_0 | kt_B | x_1 | x_2 .. x_11]
    XW = 2 * C + NG * D
    x_d = nc.declare_dram_parameter("x", [128, XW], f32, isOutput=False)
    s_d = nc.declare_dram_parameter("s", [P_N, 2], f32, isOutput=False)
    p_d = nc.declare_dram_parameter("p", [2, P_N, NF], bf16, isOutput=False)
    o_d = nc.declare_dram_parameter("o", [NG, BS, NF], bf16, isOutput=True)

    with TileContext(nc) as tc, ExitStack() as ctx:
        pool = lambda name, bufs, **kw: ctx.enter_context(
            tc.tile_pool(name=name, bufs=bufs, **kw)
        )
        const = pool("const", 1)
        rowp = pool("rowp", 2)
        small = pool("small", 2)
        wtp = pool("wtp", 3)
        obuf = pool("ob", 3)
        ps_pc = pool("ps_pc", 1, space="PSUM")
        ps_mt = pool("ps_mt", 1, space="PSUM")
        ps_o = pool("ps_o", 3, space="PSUM")

        ident = const.tile([128, 128], bf16, tag="ident")
        masks.make_identity(nc, ident[:])

        s_all = const.tile([P_N, 2], f32, tag="s")
        nc.scalar.dma_start(s_all[:], s_d[:])

        x_all = const.tile([128, XW], f32, tag="x")
        p_A = const.tile([P_N, NF], bf16, tag="pA")
        p_B = const.tile([P_N, NF], bf16, tag="pB")

        # packed-stream offsets: [kt_A | x_0 | kt_B | x_1 | x_2 .. x_11]
        ktoff = lambda lay: 0 if lay == 0 else C + D
        xoff = lambda g: C if g == 0 else 2 * C + g * D

        # ---- all loads on one HWDGE ring, in need-order ----
        def _load(c0, c1):
            nc.sync.dma_start(x_all[:, c0:c1], x_d[:, c0:c1])

        _load(0, C + D)  # kt_A + x_0
        nc.sync.dma_start(p_A[:], p_d[0])
        _load(C + D, 2 * C + 2 * D)  # kt_B + x_1
        nc.sync.dma_start(p_B[:], p_d[1])
        for k in range(5):  # x_2..x_11 in 2-group chunks
            _load(2 * C + (2 + 2 * k) * D, 2 * C + (4 + 2 * k) * D)

        # ---- selection for one group: scores -> mask -> W^T ----
        wt_tiles = {}

        def _sel(g):
            lay = _layer_of(g)
            ktc = ktoff(lay)
            xc = xoff(g)
            pc = ps_pc.tile([BS, P_N], f32, tag="pc")
            for j in range(NCH):
                nc.tensor.matmul(
                    pc[:],
                    x_all[:, xc + j * 128 : xc + (j + 1) * 128],
                    x_all[:, ktc + j * P_N : ktc + (j + 1) * P_N],
                    start=(j == 0),
                    stop=(j == NCH - 1),
                )
            mm8 = small.tile([BS, 8], f32, tag="mm8")
            nc.vector.max(mm8[:], pc[:])
            mask = rowp.tile([BS, P_N], bf16, tag="mask")
            nc.vector.tensor_scalar(
                mask[:], pc[:], mm8[:, TOP_K - 1 : TOP_K], None, OP.is_ge
            )
            mt = ps_mt.tile([P_N, BS], bf16, tag="mt")
            nc.tensor.transpose(mt[:], mask[:], ident[:])
            wt = wtp.tile([P_N, BS], bf16, tag="wt")
            nc.vector.tensor_scalar_mul(wt[:], mt[:], s_all[:, lay : lay + 1])
            wt_tiles[g] = wt

        # ---- group loop: sel runs one group ahead of the out stream ----
        _sel(0)
        for g in range(NG):
            if g + 1 < NG:
                _sel(g + 1)
            wt = wt_tiles.pop(g)
            p_sb = p_A if _layer_of(g) == 0 else p_B
            ob = obuf.tile([BS, NF], bf16, tag="ob")
            for n in range(6):
                po = ps_o.tile([BS, 1024], f32, tag="po")
                for h in range(2):
                    nc.tensor.matmul(
                        po[:, h * 512 : (h + 1) * 512],
                        wt[:],
                        p_sb[:, (2 * n + h) * 512 : (2 * n + h + 1) * 512],
                        start=True,
                        stop=True,
                    )
                if n % 3 == 2:
                    nc.vector.tensor_copy(ob[:, n * 1024 : (n + 1) * 1024], po[:])
                else:
                    nc.scalar.copy(ob[:, n * 1024 : (n + 1) * 1024], po[:])
                if n % 2 == 1:
                    t = NF // 3
                    k = n // 2
                    nc.scalar.dma_start(
                        o_d[g, :, k * t : (k + 1) * t], ob[:, k * t : (k + 1) * t]
                    )

    nc.compile()
    _CACHE["nc"] = nc
    return nc


def _pack_inputs(x_query, K_all, A_all, P_all):
    import ml_dtypes

    bf = ml_dtypes.bfloat16
    x = np.asarray(x_query, dtype=np.float32)
    K64 = np.asarray(K_all, dtype=np.float64)
    A64 = np.asarray(A_all, dtype=np.float64)
    P = np.asarray(P_all, dtype=np.float32)

    Kh64 = K64 / np.linalg.norm(K64, axis=-1, keepdims=True)
    Ah64 = A64 / np.linalg.norm(A64, axis=-1, keepdims=True)
    Kh = Kh64.astype(np.float32)
    s = np.sum(Kh64 * Ah64, axis=-1).astype(np.float32)  # [L, P]
    pp = np.ascontiguousarray(P.reshape(L, P_N, NF)).astype(bf)

    # K_hat^T per layer: [128dd, (j, p)]
    kt6 = np.ascontiguousarray(
        Kh.reshape(L, P_N, NCH, 128).transpose(0, 3, 2, 1)
    )  # [L, 128, 6, 100]

    XW = 2 * C + NG * D
    xs, ss, ps = [], [], []
    for c in range(N_CORES):
        la, lb, hb = _core_layers(c)
        ss.append(np.stack([s[la], s[lb]], axis=1))  # [P, 2]
        ps.append(np.stack([pp[la], pp[lb]], axis=0))  # [2, P, NF]
        xg = np.empty((128, NG, NCH, 128), dtype=np.float32)
        for g in range(NG):
            if g < NFULL:
                lay, r0 = la, g * BS
            else:
                lay, r0 = lb, hb * 512 + (g - NFULL) * BS
            # x rows [r0:r0+128] of layer lay -> [128dd, (j, b)]
            xg[:, g] = x[r0 : r0 + BS, lay, :].reshape(BS, NCH, 128).transpose(2, 1, 0)
        xg = xg.reshape(128, NG, D)
        # packed stream: [kt_A | x_0 | kt_B | x_1 | x_2 .. x_11]
        xp = np.empty((128, XW), dtype=np.float32)
        xp[:, :C] = kt6[la].reshape(128, C)
        xp[:, C : C + D] = xg[:, 0]
        xp[:, C + D : 2 * C + D] = kt6[lb].reshape(128, C)
        xp[:, 2 * C + D :] = xg[:, 1:].reshape(128, (NG - 1) * D)
        xs.append(xp)
    return xs, ss, ps


def _run(x_query, K_all, A_all, P_all, trace=False, tmpdir=None):
    from concourse.bass_utils import run_bass_kernel_spmd

    xs, ss, ps = _pack_inputs(x_query, K_all, A_all, P_all)
    nc = _build_nc()
    in_maps = [{"x": xs[c], "s": ss[c], "p": ps[c]} for c in range(N_CORES)]
    br = run_bass_kernel_spmd(
        nc, in_maps, list(range(N_CORES)), trace=trace, tmpdir=tmpdir
    )
    out = np.empty((L, B, NF), dtype=np.float32)
    for c in range(N_CORES):
        o = np.asarray(br.results[c]["o"]).astype(np.float32)  # [NG, BS, NF]
        la, lb, hb = _core_layers(c)
        for g in range(NG):
            if g < NFULL:
                lay, r0 = la, g * BS
            else:
                lay, r0 = lb, hb * 512 + (g - NFULL) * BS
            out[lay, r0 : r0 + BS] = o[g]
    return out.reshape(L, B, LP, D), br


def kernel(x_query, K_all, A_all, P_all):
    out, _ = _run(x_query, K_all, A_all, P_all)
    return out


# revision 29
# speedup vs baseline: 1.1724x; 1.0080x over previous
"""Trainium2 Bass kernel for nn_L2PppMaskAttn (topk_masking).

Math reformulation of the reference:
  - top-5 ranking over prompts is invariant to q normalization, so scores
    u[b,p] = <x[b,l], K_hat[l,p]> suffice; mask = (u >= 5th_max(u)).
  - a_k depends only on (layer, prompt): s[l,p] = <K_hat[l,p], A_hat[l,p]>.
  - out[l,b] = (mask_row .* s) @ P_flat[l]: a [128,100] @ [100,6144] matmul.

K_hat and s are computed at pack time in f64 and rounded to f32 once.  The
top-5 selection then reduces to 6 f32 PE matmuls per 128-row group against
the prepacked K_hat^T plus a DVE max8 threshold (comparison-only, exact).
The f64-true top-5 agrees with the reference's choice on every row of this
input set (min 5th/6th gap 9.5e-7 vs device f32 score noise ~2e-7), so the
selection reproduces the reference's mask.  The output-scale path (s, P
matmul, store) runs in bf16: ~5e-3 worst case vs the 2e-2 gate.

Sharding: work = 12 layers x 1024 rows, cut into 96 (layer, 128-row)
groups; each core takes 12 groups spanning just TWO layers (one full layer
= 8 groups + one half layer = 4 groups), so each layer's K_hat/P pool is
read by at most two cores instead of eight.  Per-core HBM traffic:
  kt    [128dd, (2 lay, j, p)]        f32   0.6 MB
  x     [128dd, (12 grp, j, b)]       f32   4.7 MB
  P     [2][100p, 6144]               bf16  2.5 MB
  out   [12 grp][128b, 6144]          bf16 18.9 MB
~26.7 MB/core vs ~42 MB for batch-only sharding (P/kt no longer 8x
replicated); the host reassembles groups into the [L, B, Lp, E] output.

Schedule: all loads ride one HWDGE ring (sync) as a few large FIFO
transfers in need-order ([kt_A|x_0], P_A, [kt_B|x_1], P_B, x pairs), so
group g's selection inputs land well before its out matmuls need P.  The
PE queue is software-pipelined: selection for group g+1 is emitted before
the 12 out matmuls of group g, and the Tile scheduler hoists all
selections into the early DMA-bound window (where the PE is still
unthrottled); keeping pc/mt in separate PSUM pools is what makes that
hoist legal.  Out matmuls write three double-bank [128,1024] PSUM tiles in
rotation; each tile is cast PSUM->SBUF bf16 (4 on scalar, 2 on vector per
group, ~1.1 us each) and each group stores as three 0.52 MB thirds on the
scalar HWDGE ring.  The PE runs at the HAM-clamped K=4/8 rate (~427 ns
per 512-col bf16 matmul) for the back half of the kernel; the schedule
keeps it >90% busy there.
"""

import sys

sys.path.insert(0, "/opt/trn_rl_repo")

import numpy as np

B, L, P_N, LP, D = 1024, 12, 100, 8, 768
N_CORES = 8
BS = 128  # rows per group
NG = 12  # groups per core
NFULL = 8  # groups 0..7 -> layer A (full batch); 8..11 -> layer B (half)
NF = LP * D  # 6144 flattened output features per layer
NCH = D // 128  # 6 contraction chunks
C = NCH * P_N  # 600 K_hat^T columns per layer
TOP_K = 5

_CACHE = {}


def _layer_of(g):
    return 0 if g < NFULL else 1


def _core_layers(c):
    # core c: full layer c, half (c % 2) of layer 8 + c // 2
    return c, 8 + c // 2, c % 2


def _build_nc():
    if "nc" in _CACHE:
        return _CACHE["nc"]

    from contextlib import ExitStack

    import concourse.bass as bass
    import concourse.bacc as bacc
    import concourse.mybir as mybir
    from concourse import masks
    from concourse.tile import TileContext

    f32 = mybir.dt.float32
    bf16 = mybir.dt.bfloat16
    OP = mybir.AluOpType

    nc = bacc.Bacc(
        "TRN2",
        target_bir_lowering=False,
        debug=False,
        num_devices=N_CORES,
    )

    # one packed stream: [kt_A | x_0 | kt_B | x_1 | x_2 .. x_11]
    XW = 2 * C + NG * D
    x_d = nc.declare_dram_parameter("x", [128, XW], f32, isOutput=False)
    s_d = nc.declare_dram_parameter("s", [P_N, 2], f32, isOutput=False)
    p_d = nc.declare_dram_parameter("p", [2, P_N, NF], bf16, isOutput=False)
    o_d = nc.declare_dram_parameter("o", [NG, BS, NF], bf16, isOutput=True)

    with TileContext(nc) as tc, ExitStack() as ctx:
        pool = lambda name, bufs, **kw: ctx.enter_context(
            tc.tile_pool(name=name, bufs=bufs, **kw)
        )
        const = pool("const", 1)
        rowp = pool("rowp", 2)
        small = pool("small", 2)
        wtp = pool("wtp", 3)
        obuf = pool("ob", 3)
        ps_pc = pool("ps_pc", 1, space="PSUM")
        ps_mt = pool("ps_mt", 1, space="PSUM")
        ps_o = pool("ps_o", 3, space="PSUM")

        ident = const.tile([128, 128], bf16, tag="ident")
        masks.make_identity(nc, ident[:])

        s_all = const.tile([P_N, 2], f32, tag="s")
        nc.scalar.dma_start(s_all[:], s_d[:])

        x_all = const.tile([128, XW], f32, tag="x")
        p_A = const.tile([P_N, NF], bf16, tag="pA")
        p_B = const.tile([P_N, NF], bf16, tag="pB")

        # packed-stream offsets: [kt_A | x_0 | kt_B | x_1 | x_2 .. x_11]
        ktoff = lambda lay: 0 if lay == 0 else C + D
        xoff = lambda g: C if g == 0 else 2 * C + g * D

        # ---- all loads on one HWDGE ring, in need-order ----
        def _load(c0, c1):
            nc.sync.dma_start(x_all[:, c0:c1], x_d[:, c0:c1])

        _load(0, C + D)  # kt_A + x_0
        nc.sync.dma_start(p_A[:], p_d[0])
        _load(C + D, 2 * C + 2 * D)  # kt_B + x_1
        nc.sync.dma_start(p_B[:], p_d[1])
        for k in range(5):  # x_2..x_11 in 2-group chunks
            _load(2 * C + (2 + 2 * k) * D, 2 * C + (4 + 2 * k) * D)

        # ---- selection for one group: scores -> mask -> W^T ----
        wt_tiles = {}

        def _sel(g):
            lay = _layer_of(g)
            ktc = ktoff(lay)
            xc = xoff(g)
            pc = ps_pc.tile([BS, P_N], f32, tag="pc")
            for j in range(NCH):
                nc.tensor.matmul(
                    pc[:],
                    x_all[:, xc + j * 128 : xc + (j + 1) * 128],
                    x_all[:, ktc + j * P_N : ktc + (j + 1) * P_N],
                    start=(j == 0),
                    stop=(j == NCH - 1),
                )
            mm8 = small.tile([BS, 8], f32, tag="mm8")
            nc.vector.max(mm8[:], pc[:])
            mask = rowp.tile([BS, P_N], bf16, tag="mask")
            nc.vector.tensor_scalar(
                mask[:], pc[:], mm8[:, TOP_K - 1 : TOP_K], None, OP.is_ge
            )
            mt = ps_mt.tile([P_N, BS], bf16, tag="mt")
            nc.tensor.transpose(mt[:], mask[:], ident[:])
            wt = wtp.tile([P_N, BS], bf16, tag="wt")
            nc.vector.tensor_scalar_mul(wt[:], mt[:], s_all[:, lay : lay + 1])
            wt_tiles[g] = wt

        # ---- group loop: sel runs one group ahead of the out stream ----
        _sel(0)
        for g in range(NG):
            if g + 1 < NG:
                _sel(g + 1)
            wt = wt_tiles.pop(g)
            p_sb = p_A if _layer_of(g) == 0 else p_B
            ob = obuf.tile([BS, NF], bf16, tag="ob")
            for n in range(6):
                po = ps_o.tile([BS, 1024], f32, tag="po")
                for h in range(2):
                    nc.tensor.matmul(
                        po[:, h * 512 : (h + 1) * 512],
                        wt[:],
                        p_sb[:, (2 * n + h) * 512 : (2 * n + h + 1) * 512],
                        start=True,
                        stop=True,
                    )
                if n % 3 == 2:
                    nc.vector.tensor_copy(ob[:, n * 1024 : (n + 1) * 1024], po[:])
                else:
                    nc.scalar.copy(ob[:, n * 1024 : (n + 1) * 1024], po[:])
                if n % 2 == 1:
                    t = NF // 3
                    k = n // 2
                    nc.scalar.dma_start(
                        o_d[g, :, k * t : (k + 1) * t], ob[:, k * t : (k + 1) * t]
                    )

    nc.compile()
    _CACHE["nc"] = nc
    return nc


def _pack_inputs(x_query, K_all, A_all, P_all):
    import ml_dtypes

    bf = ml_dtypes.bfloat16
    x = np.asarray(x_query, dtype=np.float32)
    K64 = np.asarray(K_all, dtype=np.float64)
    A64 = np.asarray(A_all, dtype=np.float64)
    P = np.asarray(P_all, dtype=np.float32)

    Kh64 = K64 / np.linalg.norm(K64, axis=-1, keepdims=True)
    Ah64 = A64 / np.linalg.norm(A64, axis=-1, keepdims=True)
    Kh = Kh64.astype(np.float32)
    s = np.sum(Kh64 * Ah64, axis=-1).astype(np.float32)  # [L, P]
    pp = np.ascontiguousarray(P.reshape(L, P_N, NF)).astype(bf)

    # K_hat^T per layer: [128dd, (j, p)]
    kt6 = np.ascontiguousarray(
        Kh.reshape(L, P_N, NCH, 128).transpose(0, 3, 2, 1)
    )  # [L, 128, 6, 100]

    XW = 2 * C + NG * D
    xs, ss, ps = [], [], []
    for c in range(N_CORES):
        la, lb, hb = _core_layers(c)
        ss.append(np.stack([s[la], s[lb]], axis=1))  # [P, 2]
        ps.append(np.stack([pp[la], pp[lb]], axis=0))  # [2, P, NF]
        xg = np.empty((128, NG, NCH, 128), dtype=np.float32)
        for g in range(NG):
            if g < NFULL:
                lay, r0 = la, g * BS
            else:
                lay, r0 = lb, hb * 512 + (g - NFULL) * BS
            # x rows [r0:r0+128] of layer lay -> [128dd, (j, b)]
            xg[:, g] = x[r0 : r0 + BS, lay, :].reshape(BS, NCH, 128).transpose(2, 1, 0)
        xg = xg.reshape(128, NG, D)
        # packed stream: [kt_A | x_0 | kt_B | x_1 | x_2 .. x_11]
        xp = np.empty((128, XW), dtype=np.float32)
        xp[:, :C] = kt6[la].reshape(128, C)
        xp[:, C : C + D] = xg[:, 0]
        xp[:, C + D : 2 * C + D] = kt6[lb].reshape(128, C)
        xp[:, 2 * C + D :] = xg[:, 1:].reshape(128, (NG - 1) * D)
        xs.append(xp)
    return xs, ss, ps


def _run(x_query, K_all, A_all, P_all, trace=False, tmpdir=None):
    from concourse.bass_utils import run_bass_kernel_spmd

    xs, ss, ps = _pack_inputs(x_query, K_all, A_all, P_all)
    nc = _build_nc()
    in_maps = [{"x": xs[c], "s": ss[c], "p": ps[c]} for c in range(N_CORES)]
    br = run_bass_kernel_spmd(
        nc, in_maps, list(range(N_CORES)), trace=trace, tmpdir=tmpdir
    )
    out = np.empty((L, B, NF), dtype=np.float32)
    for c in range(N_CORES):
        o = np.asarray(br.results[c]["o"]).astype(np.float32)  # [NG, BS, NF]
        la, lb, hb = _core_layers(c)
        for g in range(NG):
            if g < NFULL:
                lay, r0 = la, g * BS
            else:
                lay, r0 = lb, hb * 512 + (g - NFULL) * BS
            out[lay, r0 : r0 + BS] = o[g]
    return out.reshape(L, B, LP, D), br


def kernel(x_query, K_all, A_all, P_all):
    out, _ = _run(x_query, K_all, A_all, P_all)
    return out
